# revision 64
# baseline (speedup 1.0000x reference)
"""Trainium2 Bass kernel for MHA block (LN -> QKV -> qk-LN -> RoPE -> masked attn -> out-proj).

Self-contained: hardcodes shapes B=2, L=2048, D=1024, H=16, Dh=64; runs on 8 NeuronCores
via bass_utils.run_bass_kernel_spmd. Sharding: core c = (batch b = c//4, head-group
g = c%4 of 4 heads). Weight columns are sliced per core so "our" 4 heads are always
columns 0:256 -> the device program is identical on all cores (SPMD).

v2 schedule (vs the first working version):
- x is transposed by the DMA engines (XBAR dma_start_transpose) straight from DRAM,
  so the PE's phase 1 is pure QKV matmuls that start as soon as the first xT tile and
  weight chunk land (~2us). Plain x tiles are DMA'd separately only for LN1 stats.
- LN1 is folded into the psum drain: q/k keep only the -mu1*colsum(Wf) correction
  (the rstd1 factor cancels inside the downstream qk-LN; the eps shift is O(1e-5)),
  v gets the full rstd1*(v_ps - mu1*s_v) affine. No h tensor is ever materialized.
- q|k share one 512-wide psum accumulation chain per tile.
- A dummy 8-byte AllReduce at kernel start absorbs cross-core launch skew; the two
  stats AllReduces (halves of the 16 token tiles) launch right after tile 7 / tile 15
  drain so their ~25us transport latency hides under QKV / phase-A attention.
- The attention mask is folded in as 5 extra contraction rows (seq_id one-hot rows
  scaled by 8 -> masked pairs get -64 before exp). seq_id sorted -> block-diagonal;
  host computes per-256-query-group key-tile ranges, kernel only emits those units.
- Out-projection runs per qgroup (2 token tiles), staggered one qgroup behind the
  attention emission, so output DMA streams out during phase B instead of at the end.
"""

import numpy as np
import ml_dtypes
from contextlib import ExitStack

import concourse.bass as bass
import concourse.tile as tile
from concourse import bacc, mybir
from concourse import bass_utils

F32 = mybir.dt.float32
BF16 = mybir.dt.bfloat16
AF = mybir.ActivationFunctionType
ALU = mybir.AluOpType

B, L, D = 2, 2048, 1024
H, DH = 16, 64
HPC = 4          # heads per core
CD = HPC * DH    # ctx dims per core = 256
P = 128
TT = L // P      # 16 token tiles
KC = D // P      # 8 contraction chunks
QG = 256         # query group width for block-sparse attention
NG = L // QG     # 8 query groups
EPS = 1e-5
ROPE_BASE = 10000.0
MASK_A = 8.0     # mask row scale; mask bias = -MASK_A^2 = -64 for masked pairs
KR = DH + 5      # contraction rows for scores (64 dims + 5 mask rows)
VB = DH + 1      # v block width (64 dims + ones col)
RG = [[0, 1, 2, 3], [4, 5, 6, 7]]

import os
# bisect flags (1 = use the new fast path)
F_TTR = os.environ.get("F_TTR", "1") == "1"        # x^2 square+reduce (vs bn_stats)
F_ONES = os.environ.get("F_ONES", "1") == "1"      # mean via ones-col in wv
F_BF16ROPE = os.environ.get("F_BF16ROPE", "1") == "1"  # bf16 rope + gpsimd add
F_PECORR = os.environ.get("F_PECORR", "1") == "1"  # LN1 corr + qk-s1 via PE matmuls


def _bcast_free(ap, n, axis):
    """Insert a step-0 free dim of size n at position `axis` (after partition dim)."""
    new = list(ap.ap)
    new.insert(axis, [0, n])
    return bass.AP(tensor=ap.tensor, offset=ap.offset, ap=new)


def _build_units(qgr):
    """Per qgroup: phase-A key tiles (q in first half, keys < TT//2) and phase-B
    key tiles, plus the B accumulation mode ('add' if an A part exists)."""
    ua, ub, bmode = [], [], []
    for g in range(NG):
        lo, hi = qgr[g]
        kts = list(range(lo, hi))
        if g < NG // 2:
            ka = [k for k in kts if k < TT // 2]
            kb = [k for k in kts if k >= TT // 2]
        else:
            ka, kb = [], kts
        ua.append(ka)
        ub.append(kb)
        bmode.append('add' if ka else 'copy')
    return ua, ub, bmode


def build_bass(use_ln1b=False, use_qlw=False, use_klw=False,
               qgr=tuple((0, TT) for _ in range(NG))):
    nc = bacc.Bacc("TRN2", target_bir_lowering=False, debug=False, num_devices=8)
    use_lw = use_qlw or use_klw

    # ---- DRAM I/O ----
    x_d = nc.dram_tensor("x", [L, D], BF16, kind="ExternalInput").ap()
    wqk_d = nc.dram_tensor("wqk", [P, KC, 2 * CD], BF16, kind="ExternalInput").ap()
    # wv appended columns: CD=ones (sum_d x), CD+1/CD+2 = rowsum(Wq)/rowsum(Wk)
    # (per-token raw q/k sums for the qk-LN stats) when F_PECORR.
    VW = CD + 3 if F_PECORR else (CD + 1 if F_ONES else CD)
    wv_d = nc.dram_tensor("wv", [P, KC, VW], BF16, kind="ExternalInput").ap()
    wo_d = nc.dram_tensor("wo", [P, CD // P, D], BF16, kind="ExternalInput").ap()
    sqk_d = nc.dram_tensor("sqk", [1, 2 * CD], BF16, kind="ExternalInput").ap()
    sv_d = nc.dram_tensor("sv", [1, VW if F_PECORR else CD], BF16,
                          kind="ExternalInput").ap()
    mq_d = nc.dram_tensor("maskq", [5, L], BF16, kind="ExternalInput").ap()
    mk_d = nc.dram_tensor("maskk", [5, L], BF16, kind="ExternalInput").ap()
    cos_d = nc.dram_tensor("cos", [P, TT, DH], BF16, kind="ExternalInput").ap()
    sinl_d = nc.dram_tensor("sinl", [P, TT, DH // 2], BF16, kind="ExternalInput").ap()
    sinh_d = nc.dram_tensor("sinh", [P, TT, DH // 2], BF16, kind="ExternalInput").ap()
    r1_d = nc.dram_tensor("r1", [P, TT, DH], BF16, kind="ExternalInput").ap()
    idb_d = nc.dram_tensor("identb", [P, P], BF16, kind="ExternalInput").ap()
    if use_ln1b:
        bwqk_d = nc.dram_tensor("bwqk", [1, 2 * CD], F32, kind="ExternalInput").ap()
        bwv_d = nc.dram_tensor("bwv", [1, CD], F32, kind="ExternalInput").ap()
    if use_qlw:
        qlw_d = nc.dram_tensor("qlw", [1, CD], F32, kind="ExternalInput").ap()
    if use_klw:
        klw_d = nc.dram_tensor("klw", [1, CD], F32, kind="ExternalInput").ap()
    out_d = nc.dram_tensor("out", [L, D], BF16, kind="ExternalOutput").ap()

    x_t_d = x_d.rearrange("(n p) d -> n p d", p=P)
    out_t_d = out_d.rearrange("(n p) d -> n p d", p=P)

    units_a, units_b, bmode = _build_units(qgr)

    with tile.TileContext(nc) as tc, ExitStack() as ctx:
        cpool = ctx.enter_context(tc.tile_pool(name="cpool", bufs=1))
        small = ctx.enter_context(tc.tile_pool(name="small", bufs=4))
        dramp = ctx.enter_context(tc.tile_pool(name="dramp", bufs=1, space="DRAM"))

        # ---- dummy AllReduce: absorbs cross-core launch skew on the CC queue ----
        z0 = cpool.tile([P, 2], F32)
        nc.vector.memset(z0, 0.0)
        ib0 = dramp.tile([P, 2], F32)
        ob0 = dramp.tile([P, 2], F32)
        nc.gpsimd.dma_start(ib0, z0)
        nc.gpsimd.collective_compute(
            "AllReduce", ALU.add, replica_groups=RG,
            ins=[ib0.opt()], outs=[ob0.opt()],
        )

        identb = cpool.tile([P, P], BF16)
        nc.sync.dma_start(identb, idb_d)
        eps_ap = cpool.tile([P, 1], F32)
        nc.vector.memset(eps_ap, EPS)
        if F_PECORR:
            # colsum rows [1, n] (rank-1 LN1 correction via K=1 matmuls)
            sqk_sb = cpool.tile([1, 2 * CD], BF16)
            nc.sync.dma_start(sqk_sb, sqk_d)
            sv_sb = cpool.tile([1, VW], BF16)
            nc.sync.dma_start(sv_sb, sv_d)
        else:
            # colsum rows broadcast across partitions (correction on DVE)
            sqk_sb = cpool.tile([P, 2 * CD], BF16)
            nc.sync.dma_start(sqk_sb, sqk_d.partition_broadcast(P)[:, 0, :])
            sv_sb = cpool.tile([P, CD], BF16)
            nc.sync.dma_start(sv_sb, sv_d.partition_broadcast(P)[:, 0, :])
        if use_ln1b:
            bwqk_sb = cpool.tile([P, 2 * CD], F32)
            nc.sync.dma_start(bwqk_sb, bwqk_d.partition_broadcast(P)[:, 0, :])
            bwv_sb = cpool.tile([P, CD], F32)
            nc.sync.dma_start(bwv_sb, bwv_d.partition_broadcast(P)[:, 0, :])

        # v augmented: flat [128, TT*HPC*65 + 63]; per (kt,h) block of 65 cols
        # (64 v dims + ones col). PV reads 128 cols per block: the 63 cols past a
        # block belong to the next block -> garbage rows 65:128 in ctx psum, unread.
        pB = ctx.enter_context(tc.tile_pool(name="pB", bufs=1))
        v_sb = pB.tile([P, TT * HPC * VB + (P - VB)], BF16)
        v_blocks = v_sb[:, : TT * HPC * VB].rearrange("p (t h d) -> p t h d", t=TT, h=HPC)
        nc.gpsimd.memset(v_sb, 0.0)
        nc.gpsimd.memset(v_blocks[:, :, :, DH : DH + 1], 1.0)

        # qT/kT augmented per head: rows 0:64 = head dims (transposed), 64:69 = mask
        # rows -> scores+mask in ONE matmul over 69 contraction rows.
        qT = pB.tile([P, HPC, L], BF16)
        kT = pB.tile([P, HPC, L], BF16)
        # rope'd+LN-finalized q/k in token-major bf16, awaiting transpose
        rotb_q = pB.tile([P, TT, HPC, DH], BF16)
        rotb_k = pB.tile([P, TT, HPC, DH], BF16)
        craw_all = pB.tile([DH + 1, HPC, L], BF16)
        ctxT = pB.tile([P, CD // P, L], BF16)
        wo_sb = pB.tile([P, CD // P, D], BF16)

        def rotb_at(j, t):
            return (rotb_q if j == 0 else rotb_k)[:, t, :, :]

        # ============ Phase 1: QKV from DMA-transposed x + LN1 fold + stats + rope
        with ExitStack() as phA:
            pA = phA.enter_context(tc.tile_pool(name="pA", bufs=1))
            stats_pack = pA.tile([P, TT, 2, 2], F32)
            allred = pA.tile([P, TT, 2, 2], F32)
            rot_q = pA.tile([P, TT, HPC, DH], BF16 if F_BF16ROPE else F32)
            rot_k = pA.tile([P, TT, HPC, DH], BF16 if F_BF16ROPE else F32)

            def rot_at(j, t):
                return (rot_q if j == 0 else rot_k)[:, t, :, :]

            wqk_sb = pA.tile([P, KC, 2 * CD], BF16)
            wv_sb = pA.tile([P, KC, VW], BF16)
            r1_sb = pA.tile([P, TT, DH], BF16)
            if use_lw:
                qk4_all = pA.tile([P, TT, 2 * CD], F32)
                cos2_sb = pA.tile([P, TT, DH], BF16)
                nc.sync.dma_start(cos2_sb, cos_d)
                sinl2_sb = pA.tile([P, TT, DH // 2], BF16)
                nc.sync.dma_start(sinl2_sb, sinl_d)
                sinh2_sb = pA.tile([P, TT, DH // 2], BF16)
                nc.sync.dma_start(sinh2_sb, sinh_d)
                if use_qlw:
                    qlw_sb = pA.tile([P, CD], F32)
                    nc.sync.dma_start(qlw_sb, qlw_d.partition_broadcast(P)[:, 0, :])
                if use_klw:
                    klw_sb = pA.tile([P, CD], F32)
                    nc.sync.dma_start(klw_sb, klw_d.partition_broadcast(P)[:, 0, :])
            ib1 = dramp.tile([P, TT * 2], F32)
            ob1 = dramp.tile([P, TT * 2], F32)
            ib2 = dramp.tile([P, TT * 2], F32)
            ob2 = dramp.tile([P, TT * 2], F32)

            def finalize_apply(lo, hi, after_group=None, tr_engine=0):
                """qk-LN: mu/rstd from all-reduced sums, fold into raw-rope'd q/k.
                rotb = rstd * (rot + (-mu) * r1): one DVE STT + one ACT scale per
                (tile, q/k). after_group(g4) fires once each 4-tile group is done."""
                n = hi - lo
                mu = small.tile([P, n, 2], F32, tag="fmu")
                nc.vector.tensor_scalar(mu, allred[:, lo:hi, :, 0], 1.0 / D, None, ALU.mult)
                negmu = small.tile([P, n, 2], F32, tag="fnegmu")
                nc.vector.tensor_scalar(negmu, mu, -1.0, None, ALU.mult)
                m2 = small.tile([P, n, 2], F32, tag="fm2")
                nc.vector.tensor_mul(m2, mu, mu)
                rstd = small.tile([P, n, 2], F32, tag="frstd")
                nc.vector.scalar_tensor_tensor(
                    rstd, allred[:, lo:hi, :, 1], 1.0 / D, m2,
                    op0=ALU.mult, op1=ALU.subtract,
                )
                # fused 1/sqrt(var + eps) in one ACT op
                nc.scalar.activation(rstd, rstd, AF.Abs_reciprocal_sqrt, bias=eps_ap)
                nm = small.tile([P, n, 2], F32, tag="fnm")
                if use_lw:
                    nc.vector.scalar_tensor_tensor(nm, mu, -1.0, rstd, op0=ALU.mult, op1=ALU.mult)
                for t, j in [(g4s + dt, jj) for g4s in range(lo, hi, 4)
                             for jj in range(2) for dt in range(4)]:
                    lw_flag = use_qlw if j == 0 else use_klw
                    if lw_flag:
                        src4 = qk4_all[:, t, j * CD : (j + 1) * CD]
                        lw_sb = qlw_sb if j == 0 else klw_sb
                        xn = small.tile([P, HPC, DH], F32, tag="xnf")
                        nc.scalar.activation(
                            xn.rearrange("p h d -> p (h d)"), src4,
                            AF.Identity, bias=nm[:, t - lo, j : j + 1],
                            scale=rstd[:, t - lo, j : j + 1],
                        )
                        nc.vector.tensor_mul(
                            xn, xn, lw_sb.rearrange("p (h d) -> p h d", h=HPC)
                        )
                        qa2 = small.tile([P, HPC, DH], F32, tag="qa2")
                        nc.vector.tensor_mul(
                            qa2, xn, _bcast_free(cos2_sb[:, t, :], HPC, 1)
                        )
                        qb2 = small.tile([P, HPC, DH], F32, tag="qb2")
                        nc.vector.tensor_mul(
                            qb2[:, :, 0 : DH // 2], xn[:, :, DH // 2 : DH],
                            _bcast_free(sinl2_sb[:, t, :], HPC, 1),
                        )
                        nc.vector.tensor_mul(
                            qb2[:, :, DH // 2 : DH], xn[:, :, 0 : DH // 2],
                            _bcast_free(sinh2_sb[:, t, :], HPC, 1),
                        )
                        nc.vector.tensor_add(rotb_at(j, t), qa2, qb2)
                    else:
                        tmp = small.tile([P, HPC, DH], F32, tag=f"tmp{j}", bufs=2)
                        nc.vector.scalar_tensor_tensor(
                            tmp, _bcast_free(r1_sb[:, t, :], HPC, 1),
                            negmu[:, t - lo, j : j + 1], rot_at(j, t),
                            op0=ALU.mult, op1=ALU.add,
                        )
                        nc.scalar.activation(
                            rotb_at(j, t).rearrange("p h d -> p (h d)"),
                            tmp.rearrange("p h d -> p (h d)"),
                            AF.Identity, scale=rstd[:, t - lo, j : j + 1],
                        )
                    if after_group is not None and t % 4 == 3 and j == 1:
                        after_group(t // 4, tr_engine)

            with ExitStack() as pctx:
                pp_qkv = pctx.enter_context(tc.tile_pool(name="pp_qkv", bufs=2, space="PSUM"))
                pp_ht = pctx.enter_context(tc.tile_pool(name="pp_ht", bufs=2, space="PSUM"))
                p1 = pctx.enter_context(tc.tile_pool(name="p1", bufs=2))

                # x0 first (gates the first PE transpose), then weights
                xs_pre = []
                for t in range(2):
                    x_t = p1.tile([P, D], BF16, tag="x_t", bufs=8, name=f"xs{t}")
                    nc.sync.dma_start(x_t, x_t_d[t])
                    xs_pre.append(x_t)
                nc.sync.dma_start(wqk_sb[:, 0:2, :], wqk_d[:, 0:2, :])
                for t in range(2, 4):
                    x_t = p1.tile([P, D], BF16, tag="x_t", bufs=8, name=f"xs{t}")
                    nc.sync.dma_start(x_t, x_t_d[t])
                    xs_pre.append(x_t)
                nc.sync.dma_start(wqk_sb[:, 2:5, :], wqk_d[:, 2:5, :])
                nc.sync.dma_start(wqk_sb[:, 5:8, :], wqk_d[:, 5:8, :])
                nc.sync.dma_start(wv_sb[:, 0:4, :], wv_d[:, 0:4, :])
                nc.sync.dma_start(wv_sb[:, 4:8, :], wv_d[:, 4:8, :])
                cos_sb = p1.tile([P, TT, DH], BF16, bufs=1)
                nc.sync.dma_start(cos_sb, cos_d)
                sinl_sb = p1.tile([P, TT, DH // 2], BF16, bufs=1)
                nc.sync.dma_start(sinl_sb, sinl_d)
                sinh_sb = p1.tile([P, TT, DH // 2], BF16, bufs=1)
                nc.sync.dma_start(sinh_sb, sinh_d)
                nc.sync.dma_start(r1_sb, r1_d)
                # mask rows + out-proj weights early: the sync queue blocks later
                # on the allreduce-output DMAs, these must not sit behind that
                for hh in range(HPC):
                    nc.sync.dma_start(qT[DH : DH + 5, hh, :], mq_d)
                    nc.sync.dma_start(kT[DH : DH + 5, hh, :], mk_d)
                for t in range(4, TT):
                    x_t = p1.tile([P, D], BF16, tag="x_t", bufs=8, name=f"xs{t}")
                    nc.sync.dma_start(x_t, x_t_d[t])
                    xs_pre.append(x_t)
                nc.sync.dma_start(wo_sb, wo_d)

                def stage_xpose(t):
                    """PE-transpose raw x tile t (no LN dependency)."""
                    xt_ps = pp_ht.tile([P, KC, P], BF16, tag="ht")
                    for c in range(KC):
                        nc.tensor.transpose(
                            xt_ps[:, c, :], xs_pre[t][:, c * P : (c + 1) * P], identb
                        )
                    xT_t = p1.tile([P, KC, P], BF16, tag="xT", bufs=5, name=f"xT{t}")
                    nc.scalar.copy(xT_t[:, 0:4, :], xt_ps[:, 0:4, :])
                    nc.vector.tensor_copy(xT_t[:, 4:8, :], xt_ps[:, 4:8, :])
                    return xT_t

                def stage_qkv(t, xT_t):
                    """QKV matmuls; v_ps col CD accumulates sum_d x (ones col).
                    With F_PECORR, -mu1 is transposed to a [1,128] row and two
                    K=1 matmuls add -mu1*colsum(W) into both psums."""
                    qk_ps = pp_qkv.tile([P, 2 * CD], F32, tag="qk", bufs=3,
                                        name=f"qk_ps{t}")
                    v_ps = pp_qkv.tile([P, VW], F32, tag="v", bufs=3,
                                       name=f"v_ps{t}")
                    for c in range(KC):
                        nc.tensor.matmul(qk_ps, xT_t[:, c, :], wqk_sb[:, c, :],
                                         start=(c == 0), stop=(c == KC - 1))
                    for c in range(KC):
                        nc.tensor.matmul(v_ps, xT_t[:, c, :], wv_sb[:, c, :],
                                         start=(c == 0), stop=(c == KC - 1))
                    negmu = small.tile([P, 1], F32, tag="negmu")
                    if not F_PECORR:
                        return [t, qk_ps, v_ps, negmu]
                    negmub = small.tile([P, 1], BF16, tag="negmub")
                    nc.vector.tensor_scalar(
                        negmub, v_ps[:, CD : CD + 1], -1.0 / D, None, ALU.mult
                    )
                    nc.vector.tensor_copy(negmu, negmub)
                    nmt_ps = pp_ht.tile([1, P], BF16, tag="ht", name=f"nmt{t}")
                    nc.tensor.transpose(nmt_ps, negmub, identb)
                    nmt = small.tile([1, P], BF16, tag="nmts", bufs=3)
                    nc.scalar.copy(nmt, nmt_ps)
                    return [t, qk_ps, v_ps, negmu, nmt]

                def emit_corr(st):
                    """K=1 rank-1 LN1 correction matmuls; emitted one tile behind
                    the chains so the PE never waits on the nmt copy."""
                    if not F_PECORR:
                        return
                    t, qk_ps, v_ps, negmu, nmt = st
                    nc.tensor.matmul(qk_ps, nmt, sqk_sb, start=False, stop=True,
                                     skip_group_check=True)
                    nc.tensor.matmul(v_ps, nmt, sv_sb, start=False,
                                     stop=True, skip_group_check=True)

                def stage_drain(st):
                    """mu/rstd1, drains (psum pre-corrected when F_PECORR),
                    qk-LN partials, raw rope."""
                    t, qk_ps, v_ps, negmu = st[:4]
                    x_t = xs_pre[t]
                    if F_ONES and not F_PECORR:
                        nc.vector.tensor_scalar(
                            negmu, v_ps[:, CD : CD + 1], -1.0 / D, None, ALU.mult
                        )
                    if F_TTR:
                        # var1 = sum(x^2)/D - mu1^2 ; rstd1 scales only v.
                        # x^2 on gpsimd (no scalar ACT-table thrash), sum on DVE.
                        x2 = p1.tile([P, D], BF16, tag="x2", bufs=2)
                        nc.gpsimd.tensor_mul(x2, x_t, x_t)
                        s2x = small.tile([P, 1], F32, tag="s2x")
                        nc.vector.tensor_reduce(
                            s2x, x2, mybir.AxisListType.X, ALU.add,
                        )
                        m2 = small.tile([P, 1], F32, tag="m2x")
                        nc.vector.tensor_mul(m2, negmu, negmu)
                        xvar = small.tile([P, 1], F32, tag="xvar")
                        nc.vector.scalar_tensor_tensor(
                            xvar, s2x, 1.0 / D, m2, op0=ALU.mult, op1=ALU.subtract,
                        )
                    else:
                        xstats = small.tile([P, 2, 6], F32, tag="xstats")
                        for s in range(2):
                            nc.vector.bn_stats(
                                xstats[:, s, :],
                                x_t[:, s * 512 : (s + 1) * 512].rearrange(
                                    "p (s d) -> p s d", s=1
                                ),
                            )
                        xmv = small.tile([P, 2], F32, tag="xmv")
                        nc.vector.bn_aggr(xmv, xstats)
                        xvar = xmv[:, 1:2]
                        if not F_ONES:
                            nc.vector.tensor_scalar(
                                negmu, xmv[:, 0:1], -1.0, None, ALU.mult
                            )
                    xrstd = small.tile([P, 1], F32, tag="xrstd")
                    nc.scalar.activation(
                        xrstd, xvar, AF.Abs_reciprocal_sqrt, bias=eps_ap
                    )

                    # qk drain
                    if use_lw:
                        qk4 = qk4_all[:, t, :]
                    else:
                        qk4 = p1.tile([P, 2 * CD], BF16, tag="qk4", bufs=3)
                    if F_PECORR:
                        nc.scalar.copy(qk4, qk_ps)
                    else:
                        nc.vector.scalar_tensor_tensor(
                            qk4, sqk_sb, negmu, qk_ps, op0=ALU.mult, op1=ALU.add,
                        )
                    if use_ln1b:
                        xstd = small.tile([P, 1], F32, tag="xstd")
                        nc.vector.reciprocal(xstd, xrstd)
                        nc.vector.scalar_tensor_tensor(
                            qk4, bwqk_sb, xstd, qk4, op0=ALU.mult, op1=ALU.add,
                        )
                    # v drain: rstd1 scale into the bf16 v blocks
                    if F_PECORR and not use_ln1b:
                        nc.scalar.activation(
                            v_blocks[:, t, :, 0:DH],
                            v_ps[:, 0:CD].rearrange("p (h d) -> p h d", h=HPC),
                            AF.Identity, scale=xrstd,
                        )
                    else:
                        vtmp = small.tile([P, CD], F32, tag="vtmp", bufs=2)
                        if F_PECORR:
                            nc.scalar.activation(vtmp, v_ps[:, 0:CD], AF.Copy)
                        else:
                            nc.vector.scalar_tensor_tensor(
                                vtmp, sv_sb, negmu, v_ps[:, 0:CD],
                                op0=ALU.mult, op1=ALU.add,
                            )
                        if use_ln1b:
                            nc.scalar.activation(vtmp, vtmp, AF.Identity, scale=xrstd)
                            nc.vector.tensor_add(
                                v_blocks[:, t, :, 0:DH],
                                vtmp.rearrange("p (h d) -> p h d", h=HPC),
                                bwv_sb.rearrange("p (h d) -> p h d", h=HPC),
                            )
                        else:
                            nc.scalar.activation(
                                v_blocks[:, t, :, 0:DH],
                                vtmp.rearrange("p (h d) -> p h d", h=HPC),
                                AF.Identity, scale=xrstd,
                            )

                    # partial qk-LN sums: s1 from the rowsum columns (PE),
                    # s2 = sum(q^2) via gpsimd square + DVE free-axis reduce
                    if F_PECORR and not use_ln1b:
                        nc.vector.tensor_copy(
                            stats_pack[:, t, :, 0],
                            v_ps[:, CD + 1 : CD + 3],
                        )
                        q2 = p1.tile([P, 2 * CD], BF16, tag="q2", bufs=2)
                        nc.gpsimd.tensor_mul(q2, qk4, qk4)
                        nc.vector.tensor_reduce(
                            stats_pack[:, t, :, 1],
                            q2.rearrange("p (j d) -> p j d", j=2),
                            mybir.AxisListType.X, ALU.add,
                        )
                    else:
                        # generic: bn stats; s1 = 256*mean, s2 = 256*(var+mean^2)
                        for j in range(2):
                            src4 = qk4[:, j * CD : (j + 1) * CD]
                            qs6 = small.tile([P, 1, 6], F32, tag=f"qs6{j}")
                            nc.vector.bn_stats(qs6, src4.rearrange("p (s d) -> p s d", s=1))
                            qmv = small.tile([P, 2], F32, tag=f"qmv{j}")
                            nc.vector.bn_aggr(qmv, qs6)
                            m2s = small.tile([P, 1], F32, tag=f"m2s{j}")
                            nc.vector.tensor_scalar(
                                m2s, qmv[:, 0:1], qmv[:, 0:1], float(CD),
                                ALU.mult, ALU.mult,
                            )
                            nc.vector.tensor_scalar(
                                stats_pack[:, t, j, 0:1], qmv[:, 0:1], float(CD),
                                None, ALU.mult,
                            )
                            nc.vector.scalar_tensor_tensor(
                                stats_pack[:, t, j, 1:2], qmv[:, 1:2], float(CD), m2s,
                                op0=ALU.mult, op1=ALU.add,
                            )

                    if use_lw:
                        return
                    # raw rope (linear; LN affine folded in afterwards)
                    RDT = BF16 if F_BF16ROPE else F32
                    qk4v = qk4.rearrange("p (j h d) -> p j h d", j=2, h=HPC)
                    for j in range(2):
                        xn4 = qk4v[:, j]
                        qa = p1.tile([P, HPC, DH], RDT, tag="qa", bufs=2)
                        nc.vector.tensor_mul(qa, xn4, _bcast_free(cos_sb[:, t, :], HPC, 1))
                        qb = p1.tile([P, HPC, DH], RDT, tag="qb", bufs=2)
                        nc.gpsimd.tensor_mul(
                            qb[:, :, 0 : DH // 2],
                            xn4[:, :, DH // 2 : DH],
                            _bcast_free(sinl_sb[:, t, :], HPC, 1),
                        )
                        nc.gpsimd.tensor_mul(
                            qb[:, :, DH // 2 : DH],
                            xn4[:, :, 0 : DH // 2],
                            _bcast_free(sinh_sb[:, t, :], HPC, 1),
                        )
                        nc.vector.tensor_add(rot_at(j, t), qa, qb)

                # software pipeline: transposes run 3 tiles ahead of the QKV
                # matmuls; stage_drain(t-1) overlaps stage_qkv(t).
                xTs = {t: stage_xpose(t) for t in range(3)}
                st_prev = None
                for t in range(TT):
                    st = stage_qkv(t, xTs.pop(t))
                    if t + 3 < TT:
                        xTs[t + 3] = stage_xpose(t + 3)
                    if st_prev is not None:
                        emit_corr(st_prev)
                        stage_drain(st_prev)
                        if t == 8:
                            # first-half AllReduce; launched as soon as tiles 0-7
                            # stats exist, lands mid phase 1. All on the sync
                            # queue (gpsimd is backlogged with drain work).
                            nc.gpsimd.dma_start(
                                ib1[:],
                                stats_pack[:, 0:8, :, :].rearrange("p t j s -> p (t j s)"),
                            )
                            nc.gpsimd.collective_compute(
                                "AllReduce", ALU.add, replica_groups=RG,
                                ins=[ib1.opt()], outs=[ob1.opt()],
                            )
                            nc.sync.dma_start(
                                allred[:, 0:8, :, :].rearrange("p t j s -> p (t j s)"),
                                ob1[:],
                            )
                    st_prev = st
                emit_corr(st_prev)
                stage_drain(st_prev)

            # second-half AllReduce launched immediately after last drain
            nc.gpsimd.dma_start(
                ib2[:],
                stats_pack[:, 8:16, :, :].rearrange("p t j s -> p (t j s)"),
            )
            nc.gpsimd.collective_compute(
                "AllReduce", ALU.add, replica_groups=RG,
                ins=[ib2.opt()], outs=[ob2.opt()],
            )
            nc.sync.dma_start(
                allred[:, 8:16, :, :].rearrange("p t j s -> p (t j s)"),
                ob2[:],
            )

            # ============ Phase 2: attention (A under AR2, then B), outproj per qgroup
            with ExitStack() as actx:
                pa_sc = actx.enter_context(tc.tile_pool(name="pa_sc", bufs=3, space="PSUM"))
                pa_ctx = actx.enter_context(tc.tile_pool(name="pa_ctx", bufs=3, space="PSUM"))
                pp_tr = actx.enter_context(tc.tile_pool(name="pp_tr", bufs=2, space="PSUM"))
                p2 = actx.enter_context(tc.tile_pool(name="p2", bufs=4))

                def emit_transposes_group(g4, tr_engine):
                    for j in range(2):
                        dst = qT if j == 0 else kT
                        for hh in range(HPC):
                            tp = pp_tr.tile([DH, 4, P], BF16, tag="tp",
                                            name=f"tp{j}{hh}{g4}")
                            for i in range(4):
                                nc.tensor.transpose(
                                    tp[:, i, :], rotb_at(j, g4 * 4 + i)[:, hh, :],
                                    identb,
                                )
                            dsl = dst[0:DH, hh, g4 * 512 : (g4 + 1) * 512]
                            tpf = tp.rearrange("p g t -> p (g t)")
                            # gpsimd can't read PSUM: scalar+vector split
                            if hh % 2 == 0:
                                nc.scalar.copy(dsl, tpf)
                            else:
                                nc.vector.tensor_copy(dsl, tpf)

                def emit_attn(units, close_engine=0):
                    # units: list of (h, g, kts, mode); chunks of <=2 key tiles,
                    # software-pipelined (depth 3) scores->exp->PV
                    chunks = []
                    for ui, (h, g, kts, mode) in enumerate(units):
                        for ci in range(0, len(kts), 2):
                            sub = kts[ci : ci + 2]
                            chunks.append({
                                "h": h, "g": g, "kts": sub, "mode": mode, "ui": ui,
                                "first": ci == 0, "last": ci + 2 >= len(kts),
                            })
                    cur_ctx = [None]

                    def emit_pv(c):
                        nk = len(c["kts"])
                        for i, kt in enumerate(c["kts"]):
                            voff = (kt * HPC + c["h"]) * VB
                            nc.tensor.matmul(
                                c["ctx"], v_sb[:, voff : voff + P],
                                c["eT"][:, i * QG : (i + 1) * QG],
                                start=(c["first"] and i == 0),
                                stop=(c["last"] and i == nk - 1),
                            )
                        if c["last"]:
                            dst = craw_all[:, c["h"], c["g"] * QG : (c["g"] + 1) * QG]
                            if c["mode"] == "add":
                                nc.vector.tensor_add(dst, dst, c["ctx"][0 : DH + 1, :])
                            elif close_engine == 0 or c["h"] % 2:
                                nc.vector.tensor_copy(dst, c["ctx"][0 : DH + 1, :])
                            else:
                                nc.scalar.copy(dst, c["ctx"][0 : DH + 1, :])

                    pend = []
                    for c in chunks:
                        nk = len(c["kts"])
                        s_ps = pa_sc.tile([P, 2 * QG], F32, tag="sc", name="s_ps")
                        for i, kt in enumerate(c["kts"]):
                            nc.tensor.matmul(
                                s_ps[:, i * QG : (i + 1) * QG],
                                kT[0:KR, c["h"], kt * P : (kt + 1) * P],
                                qT[0:KR, c["h"], c["g"] * QG : (c["g"] + 1) * QG],
                                start=True, stop=True,
                            )
                        eT = p2.tile([P, 2 * QG], BF16, tag="eT", name="eT")
                        nc.scalar.activation(
                            eT[:, : nk * QG], s_ps[:, : nk * QG], AF.Exp
                        )
                        c["eT"] = eT
                        if c["first"]:
                            cur_ctx[0] = pa_ctx.tile([P, QG], F32, tag="ctx",
                                                     name="ctx_ps", bufs=3)
                        c["ctx"] = cur_ctx[0]
                        pend.append(c)
                        if len(pend) > 3:
                            emit_pv(pend.pop(0))
                    for c in pend:
                        emit_pv(c)

                def emit_outproj_qg(g, late=False):
                    """Normalize craw for qgroup g (2 token tiles) and project out.
                    late=True -> avoid scalar (EXP-bound) for the copies."""
                    for h in range(HPC):
                        pr, sub = h // 2, h % 2
                        rows = slice(sub * DH, (sub + 1) * DH)
                        fwd = pp_tr.tile([P, 2, DH + 2], BF16, tag="tp",
                                         name=f"fwd{g}{h}")
                        for i in range(2):
                            tix = g * 2 + i
                            nc.tensor.transpose(
                                fwd[:, i, 0 : DH + 1],
                                craw_all[:, h, tix * P : (tix + 1) * P],
                                identb[: DH + 1, : DH + 1],
                            )
                        rz = small.tile([P, 2], F32, tag="rz")
                        nc.vector.reciprocal(rz, fwd[:, :, DH])
                        cn = p2.tile([P, 2, DH], BF16, tag="cn", bufs=2,
                                     name=f"cn{g}{h}")
                        nc.vector.tensor_mul(cn, fwd[:, :, 0:DH], _bcast_free(rz, DH, 2))
                        back = pp_tr.tile([DH, 2, P], BF16, tag="tp",
                                          name=f"back{g}{h}")
                        for i in range(2):
                            nc.tensor.transpose(back[:, i, :], cn[:, i, :], identb)
                        nc.vector.tensor_copy(
                            ctxT[rows, pr, g * 2 * P : (g + 1) * 2 * P],
                            back.rearrange("p g t -> p (g t)"),
                        )
                    for i in range(2):
                        t = g * 2 + i
                        o_ps = [pa_ctx.tile([P, 512], F32, tag="ctx", bufs=3,
                                            name=f"o_ps{t}{s}")
                                for s in range(2)]
                        for s in range(2):
                            for c in range(CD // P):
                                nc.tensor.matmul(
                                    o_ps[s], ctxT[:, c, t * P : (t + 1) * P],
                                    wo_sb[:, c, s * 512 : (s + 1) * 512],
                                    start=(c == 0), stop=(c == CD // P - 1),
                                )
                        o_t = p2.tile([P, D], BF16, tag="o_t", bufs=2, name=f"o_t{t}")
                        nc.scalar.copy(o_t[:, 0:512], o_ps[0])
                        nc.vector.tensor_copy(o_t[:, 512:1024], o_ps[1])
                        if late:
                            # sync queue is free once AR2's output landed
                            nc.sync.dma_start(out_t_d[t], o_t)
                        else:
                            # scalar issues this one: sync may still be blocked
                            # on the AR2 output wait
                            nc.scalar.dma_start(out_t_d[t], o_t)

                # --- phase A: first-half qgroups x first-half keys, while AR2 flies
                finalize_apply(0, 8, after_group=emit_transposes_group, tr_engine=0)
                a_closed = [g for g in range(NG // 2) if units_a[g] and not units_b[g]]
                pend_op = []
                for g in range(NG // 2):
                    if units_a[g]:
                        emit_attn([(h, g, units_a[g], 'copy') for h in range(HPC)])
                        if g in a_closed:
                            pend_op.append(g)
                    if len(pend_op) > 1:
                        emit_outproj_qg(pend_op.pop(0), late=False)
                for g in pend_op:
                    emit_outproj_qg(g, late=False)

                # --- phase B
                finalize_apply(8, 16, after_group=emit_transposes_group, tr_engine=1)
                border = [g for g in range(NG) if units_b[g]]
                prev = None
                for g in border:
                    emit_attn([(h, g, units_b[g], bmode[g]) for h in range(HPC)],
                              close_engine=1)
                    if prev is not None:
                        emit_outproj_qg(prev, late=True)
                    prev = g
                if prev is not None:
                    emit_outproj_qg(prev, late=True)

    nc.compile()
    return nc


_CACHE = {}


def _get_nc(key):
    if key not in _CACHE:
        _CACHE[key] = build_bass(*key)
    return _CACHE[key]


def _qg_ranges(seq_id):
    """Per 256-query group: key-tile range (lo, hi) needed, unioned over batches.
    Falls back to dense if any row is unsorted."""
    for b in range(seq_id.shape[0]):
        if np.any(np.diff(seq_id[b].astype(np.int64)) < 0):
            return tuple((0, TT) for _ in range(NG))
    rs = []
    for g in range(NG):
        lo, hi = L, 0
        for b in range(seq_id.shape[0]):
            s = seq_id[b]
            l = int(np.searchsorted(s, s[g * QG], 'left'))
            h_ = int(np.searchsorted(s, s[(g + 1) * QG - 1], 'right'))
            lo = min(lo, l)
            hi = max(hi, h_)
        rs.append((lo // P, -(-hi // P)))
    return tuple(rs)


def _host_prep(x, seq_id, ln1_w, ln1_b, w_qkv, q_ln_w, k_ln_w, w_out):
    """Build the 8 per-core input maps."""
    x = np.asarray(x, np.float32)
    seq_id = np.asarray(seq_id)
    ln1_w = np.asarray(ln1_w, np.float32)
    ln1_b = np.asarray(ln1_b, np.float32)
    w_qkv = np.asarray(w_qkv, np.float32)
    q_ln_w = np.asarray(q_ln_w, np.float32)
    k_ln_w = np.asarray(k_ln_w, np.float32)
    w_out = np.asarray(w_out, np.float32)

    use_ln1b = bool(np.any(ln1_b != 0.0))
    use_qlw = not np.allclose(q_ln_w, 1.0)
    use_klw = not np.allclose(k_ln_w, 1.0)
    qgr = _qg_ranges(seq_id)

    BD = ml_dtypes.bfloat16
    wq_f = (w_qkv[:, 0:D] * ln1_w[:, None]).astype(BD)
    wk_f = (w_qkv[:, D : 2 * D] * ln1_w[:, None]).astype(BD)
    wv_f = (w_qkv[:, 2 * D : 3 * D] * ln1_w[:, None]).astype(BD)
    wo_b = w_out.astype(BD)
    x_b = x.astype(BD)

    # rope tables, with 1/sqrt(sqrt(64)) on each side -> scores * 1/8
    inv_freq = 1.0 / (ROPE_BASE ** (np.arange(0, DH, 2, dtype=np.float32) / DH))
    tpos = np.arange(L, dtype=np.float32)
    freqs = np.einsum("l,f->lf", tpos, inv_freq)
    emb = np.concatenate([freqs, freqs], axis=-1)
    s8 = np.float32(8.0 ** -0.5)
    cos_t = (np.cos(emb) * s8).astype(np.float32)
    sin_t = (np.sin(emb) * s8).astype(np.float32)
    sinl = -sin_t[:, : DH // 2]
    sinh = sin_t[:, DH // 2 :]
    r1 = np.concatenate(
        [cos_t[:, : DH // 2] - sin_t[:, : DH // 2],
         cos_t[:, DH // 2 :] + sin_t[:, DH // 2 :]], axis=1
    )

    identb = np.eye(P, dtype=BD)

    def wlay(a):   # [D, n] -> [P, KC, n] with d = c*P + p
        return np.ascontiguousarray(a.reshape(KC, P, -1).transpose(1, 0, 2))

    def tlay(a):   # [L, d] -> [P, TT, d] with l = n*P + p
        return np.ascontiguousarray(a.reshape(TT, P, -1).transpose(1, 0, 2))

    in_maps = []
    for c in range(8):
        b, g = c // HPC, c % HPC
        mine = np.arange(g * CD, (g + 1) * CD)

        sid = np.asarray(seq_id[b], np.int64)
        A = (sid[None, :] == np.arange(4)[:, None]).astype(np.float32)
        maskq = np.concatenate([MASK_A * A, MASK_A * np.ones((1, L), np.float32)])
        maskk = np.concatenate([MASK_A * A, -MASK_A * np.ones((1, L), np.float32)])

        wqk_c = np.concatenate(
            [wq_f[:, mine].astype(np.float32), wk_f[:, mine].astype(np.float32)],
            axis=1,
        )
        wv_c = wv_f[:, mine].astype(np.float32)
        if F_PECORR:
            # ones col (sum x) + rowsum(Wq)/rowsum(Wk) cols (raw qk-LN s1 sums)
            wv_aug = np.concatenate(
                [wv_c, np.ones((D, 1), np.float32),
                 wqk_c[:, :CD].sum(axis=1, keepdims=True, dtype=np.float32),
                 wqk_c[:, CD:].sum(axis=1, keepdims=True, dtype=np.float32)],
                axis=1,
            )
        elif F_ONES:
            wv_aug = np.concatenate([wv_c, np.ones((D, 1), np.float32)], axis=1)
        else:
            wv_aug = wv_c
        m = {
            "x": np.ascontiguousarray(x_b[b]),
            "wqk": wlay(wqk_c.astype(BD)),
            "wv": wlay(wv_aug.astype(BD)),
            "wo": np.ascontiguousarray(
                wo_b[mine, :].reshape(CD // P, P, D).transpose(1, 0, 2)),
            "sqk": wqk_c.sum(axis=0, dtype=np.float32).reshape(1, 2 * CD).astype(BD),
            "sv": (np.concatenate(
                [wv_c.sum(axis=0, dtype=np.float32),
                 np.array([0.0, wqk_c[:, :CD].sum(dtype=np.float32),
                           wqk_c[:, CD:].sum(dtype=np.float32)], np.float32)]
            ).reshape(1, CD + 3) if F_PECORR
                else wv_c.sum(axis=0, dtype=np.float32).reshape(1, CD)).astype(BD),
            "maskq": maskq.astype(BD),
            "maskk": maskk.astype(BD),
            "cos": tlay(cos_t.astype(BD)),
            "sinl": tlay(np.ascontiguousarray(sinl).astype(BD)),
            "sinh": tlay(np.ascontiguousarray(sinh).astype(BD)),
            "r1": tlay(np.ascontiguousarray(r1).astype(BD)),
            "identb": identb,
        }
        if use_ln1b:
            m["bwqk"] = (ln1_b @ np.concatenate(
                [w_qkv[:, 0:D][:, mine], w_qkv[:, D : 2 * D][:, mine]], axis=1
            )).astype(np.float32).reshape(1, 2 * CD)
            m["bwv"] = (ln1_b @ w_qkv[:, 2 * D : 3 * D][:, mine]).astype(
                np.float32).reshape(1, CD)
        if use_qlw:
            m["qlw"] = q_ln_w[mine].reshape(1, CD)
        if use_klw:
            m["klw"] = k_ln_w[mine].reshape(1, CD)
        in_maps.append(m)
    return in_maps, (use_ln1b, use_qlw, use_klw, qgr)


def run(inputs, trace=False):
    """Run on hardware; returns (output [B, L, D] fp32, BassKernelResults)."""
    in_maps, key = _host_prep(**inputs)
    nc = _get_nc(key)
    res = bass_utils.run_bass_kernel_spmd(
        nc, in_maps, core_ids=list(range(8)), trace=trace
    )
    out = np.zeros((B, L, D), np.float32)
    for c in range(8):
        out[c // HPC] += np.asarray(res.results[c]["out"], dtype=np.float32)
    return out, res


def kernel(**inputs) -> np.ndarray:
    out, _ = run(inputs)
    return out
